# revision 2
# baseline (speedup 1.0000x reference)
"""Alignment generator (length regulator) on 8 TRN2 NeuronCores.

out[b, f, j] = 1.0  iff  starts[b,j] <= f < ends[b,j]  (ends = cumsum(dur))

Each output row out[b, f, :] is one-hot at token_id[b, f] =
searchsorted(ends[b], f, side='right') (or all-zero when no token covers
frame f). The host computes token_id from the tiny [32, 512] duration input;
each core then generates its 4-row slab of the ~256MB output with one DVE
tensor_scalar(is_equal) per [128, 512] tile and streams it out in big HWDGE
DMAs. The kernel is write-bandwidth bound (~358-390 GB/s per core to HBM).

Layout: partition p of row b covers a CONTIGUOUS frame span, so every
output DMA is linear in DRAM (20KB+ contiguous per partition-descriptor).

SDMA engine skew: descriptors for SBUF partition p are serviced by the DMA
engine owning p's AXI port (port 2j <- {4j..4j+3, 32+4j..}, port 2j+1 <-
{64+4j.., 96+4j..}). Engine 15 (partitions 92-95 and 124-127) is measurably
~19% slower than the rest, so with uniform spans it runs ~12us past the end
of the stream as a solo straggler. Fix: partitions {92..95, 124..127} get
HALF-length spans (L_e ~ L_n/2); the remaining 120 partitions absorb the
difference (15 engines x 27 GB/s > the per-core HBM cap, so no loss).
Per chunk the DMA is split into segments A [0:92], C [96:124] (full rate,
A on the SP HWDGE ring, C on the ACT ring) and B [92:96], D [124:128]
(light, ACT ring). Every DMA instruction increments its slot semaphore by
16 regardless of partition count (cf. concourse/zero.py remainder path).

Raw Bass (no Tile): this walrus build only allows a single sync-wait per
compute/DMA instruction, so all synchronization is explicit standalone
wait_ge with a ring of NBUF buffers and one completion semaphore per buffer
slot; per-slot DMA-count bookkeeping makes "slot's previous DMAs fully
drained" provable from a 16*count threshold.

Sharding: pure data parallelism, batch dim 32 -> 4 rows per core; no
collectives.
"""

import math
from contextlib import ExitStack

import numpy as np

import concourse.bass as bass
import concourse.mybir as mybir
from concourse.bass_utils import run_bass_kernel_spmd

N_CORES = 8
B = 32          # batch
T = 512         # tokens
P = 128         # SBUF partitions
GROUP = 11      # span steps per buffer slot / steady-state chunk
NBUF = 4        # output buffer ring slots

# partition groups: SLOW engine 15 owns partitions 92-95 and 124-127
NA, NB, NC_, ND = 92, 4, 28, 4          # partitions 0:92, 92:96, 96:124, 124:128

_nc_cache: dict[tuple[int, int], bass.Bass] = {}


def _geometry(m_frames: int):
    """Per-partition span lengths: 120 normal partitions get L_n frames,
    the 8 engine-15 partitions get L_e (~half)."""
    L_n = max(1, math.ceil(m_frames / 124))
    L_e = min(L_n, max(0, math.ceil((m_frames - 120 * L_n) / 8)))
    m_pad = 120 * L_n + 8 * L_e
    assert m_pad >= m_frames
    return L_n, L_e, m_pad


def _rounds(L_n: int, b_loc: int):
    """(row, first_span_step, n_span_steps). Ramp the first row's chunks
    (1,1,2,4,...) so the first output DMA is issued as soon as possible
    after the input lands -- the DMA stream is the bottleneck and every ns
    it starts earlier is a ns off the kernel."""
    rounds = []
    for b in range(b_loc):
        g0 = 0
        for g in [1, 1, 2, 4, 3] if b == 0 else []:
            if g0 + g > L_n:
                break
            rounds.append((b, g0, g))
            g0 += g
        while g0 < L_n:
            g = min(GROUP, L_n - g0)
            rounds.append((b, g0, g))
            g0 += g
    return rounds


def _build(m_frames: int, b_loc: int) -> bass.Bass:
    """Per-core Bass graph writing a [b_loc, m_pad, T] padded output slab."""
    L_n, L_e, m_pad = _geometry(m_frames)
    rounds = _rounds(L_n, b_loc)
    n_rounds = len(rounds)

    def light_g(g0, g):
        return max(0, min(g, L_e - g0))

    # DMA instructions per round: A, C always; B, D while the light span
    # (first L_e steps) overlaps this chunk.
    ndma = [2 + (2 if light_g(g0, g) > 0 else 0) for (_, g0, g) in rounds]

    nc = bass.Bass()
    # input column (b*L_n + k) on partition p = token id of frame
    # (span_start(p) + k) of row b; the iota row J (J[p,j] = j) is generated
    # on-device by GpSimd in parallel with this DMA
    tid = nc.declare_dram_parameter(
        "tid", [P, b_loc * L_n], mybir.dt.float32, isOutput=False
    )
    out = nc.declare_dram_parameter(
        "out", [b_loc, m_pad, T], mybir.dt.float32, isOutput=True
    )

    # DRAM row ranges of the four partition segments (per output row)
    rA = (0, NA * L_n)
    rB = (rA[1], rA[1] + NB * L_e)
    rC = (rB[1], rB[1] + NC_ * L_n)
    rD = (rC[1], rC[1] + ND * L_e)
    assert rD[1] == m_pad

    with ExitStack() as ctx:
        sb = ctx.enter_context(
            nc.sbuf_tensor("sb", [P, b_loc * L_n], mybir.dt.float32)
        )
        Jsb = ctx.enter_context(nc.sbuf_tensor("J", [P, T], mybir.dt.float32))
        bufs = [
            ctx.enter_context(
                nc.sbuf_tensor(f"buf{s}", [P, GROUP * T], mybir.dt.float32)
            )
            for s in range(NBUF)
        ]
        in_sem = ctx.enter_context(nc.semaphore("in_sem"))
        j_sem = ctx.enter_context(nc.semaphore("j_sem"))
        c_sem = ctx.enter_context(nc.semaphore("c_sem"))
        d_sems = [ctx.enter_context(nc.semaphore(f"d_sem{s}")) for s in range(NBUF)]
        block = ctx.enter_context(nc.Block())

        @block.gpsimd
        def _(gpsimd):
            # values 0..511 are exact in fp32
            gpsimd.iota(
                Jsb[:, :],
                pattern=[[1, T]],
                base=0,
                channel_multiplier=0,
                allow_small_or_imprecise_dtypes=True,
            ).then_inc(j_sem, 1)

        def seg_view(b, row_lo, row_hi, np_, g0, g):
            return out[b][row_lo:row_hi].rearrange("(p i) t -> p (i t)", p=np_)[
                :, g0 * T : (g0 + g) * T
            ]

        @block.sync
        def _(sync):
            sync.dma_start(out=sb[:, :], in_=tid[:, :]).then_inc(in_sem, 16)
            # segment A (partitions 0:92, full rate) on the SP HWDGE ring
            for r, (b, g0, g) in enumerate(rounds):
                s = r % NBUF
                sync.wait_ge(c_sem, r + 1)
                sync.dma_start(
                    out=seg_view(b, rA[0], rA[1], NA, g0, g),
                    in_=bufs[s][0:NA, : g * T],
                ).then_inc(d_sems[s], 16)
            # all output bytes landed before the NEFF may finish
            tot = [0] * NBUF
            for r in range(n_rounds):
                tot[r % NBUF] += ndma[r]
            for s in range(NBUF):
                if tot[s]:
                    sync.wait_ge(d_sems[s], 16 * tot[s])

        @block.scalar
        def _(scalar):
            # segments C (partitions 96:124, full rate) and B/D (the slow
            # engine's light spans) on the ACT HWDGE ring
            for r, (b, g0, g) in enumerate(rounds):
                s = r % NBUF
                gl = light_g(g0, g)
                scalar.wait_ge(c_sem, r + 1)
                scalar.dma_start(
                    out=seg_view(b, rC[0], rC[1], NC_, g0, g),
                    in_=bufs[s][NA + NB : NA + NB + NC_, : g * T],
                ).then_inc(d_sems[s], 16)
                if gl > 0:
                    scalar.dma_start(
                        out=seg_view(b, rB[0], rB[1], NB, g0, gl),
                        in_=bufs[s][NA : NA + NB, : gl * T],
                    ).then_inc(d_sems[s], 16)
                    scalar.dma_start(
                        out=seg_view(b, rD[0], rD[1], ND, g0, gl),
                        in_=bufs[s][NA + NB + NC_ :, : gl * T],
                    ).then_inc(d_sems[s], 16)

        @block.vector
        def _(vector):
            vector.wait_ge(j_sem, 1)
            vector.wait_ge(in_sem, 16)
            cum = [0] * NBUF  # DMAs issued into slot s before this round
            for r, (b, g0, g) in enumerate(rounds):
                s = r % NBUF
                if r >= NBUF:
                    # slot's previous DMAs (round r-NBUF) fully drained
                    vector.wait_ge(d_sems[s], 16 * cum[s])
                last = None
                for k in range(g):
                    col = b * L_n + g0 + k
                    last = nc.vector.tensor_scalar(
                        out=bufs[s][:, k * T : (k + 1) * T],
                        in0=Jsb[:, :],
                        scalar1=sb[:, col : col + 1],
                        scalar2=None,
                        op0=mybir.AluOpType.is_equal,
                    )
                last.then_inc(c_sem, 1)
                cum[s] += ndma[r]

    return nc


def _span_starts(L_n: int, L_e: int) -> np.ndarray:
    """DRAM frame index where partition p's span begins (length L_n for the
    120 normal partitions, L_e for engine-15's {92..95, 124..127})."""
    starts = np.empty(P, dtype=np.int64)
    for p in range(P):
        if p < 92:
            starts[p] = p * L_n
        elif p < 96:
            starts[p] = 92 * L_n + (p - 92) * L_e
        elif p < 124:
            starts[p] = 92 * L_n + 4 * L_e + (p - 96) * L_n
        else:
            starts[p] = 92 * L_n + 4 * L_e + 28 * L_n + (p - 124) * L_e
    return starts


def _token_ids(dur: np.ndarray, m_pad: int) -> np.ndarray:
    """tid[b, f] = index of the token whose frame interval contains f,
    or T (out of range -> all-zero output row) when no token covers f."""
    ends = np.cumsum(dur.astype(np.int64), axis=1)
    frames = np.arange(m_pad, dtype=np.int64)
    tid = np.empty((dur.shape[0], m_pad), dtype=np.float32)
    for b in range(dur.shape[0]):
        tid[b] = np.searchsorted(ends[b], frames, side="right")
    return tid


def _prepare(duration_predictor_output: np.ndarray, max_frames):
    """Host-side prep: token ids, per-core input maps, cached Bass graph."""
    dur = np.asarray(duration_predictor_output)
    m_frames = int(max_frames)
    b_loc = B // N_CORES
    L_n, L_e, m_pad = _geometry(m_frames)

    tid = _token_ids(dur, m_pad)  # [B, m_pad] float32
    starts = _span_starts(L_n, L_e)
    span_len = np.where((np.arange(P) >= 92) & (np.arange(P) < 96)
                        | (np.arange(P) >= 124), L_e, L_n)

    key = (m_frames, b_loc)
    nc = _nc_cache.get(key)
    if nc is None:
        nc = _build(m_frames, b_loc)
        _nc_cache[key] = nc

    # gather index [P, L_n]: frame index for (partition, span step), clamped;
    # steps beyond a light partition's span are masked to token id T
    k = np.arange(L_n)[None, :]
    idx = np.minimum(starts[:, None] + k, m_pad - 1)
    mask = k >= span_len[:, None]

    in_maps = []
    for i in range(N_CORES):
        cols = []
        for b in range(b_loc):
            tb = tid[i * b_loc + b][idx]          # [P, L_n]
            tb[mask] = float(T)
            cols.append(tb)
        in_maps.append({"tid": np.ascontiguousarray(np.concatenate(cols, axis=1))})
    return nc, in_maps


def kernel(duration_predictor_output: np.ndarray, max_frames) -> np.ndarray:
    dur = np.asarray(duration_predictor_output)
    m_frames = int(max_frames)
    if m_frames <= 0:
        return np.zeros((dur.shape[0], 0, dur.shape[1]), dtype=np.float32)

    nc, in_maps = _prepare(dur, m_frames)
    res = run_bass_kernel_spmd(nc, in_maps, core_ids=list(range(N_CORES)))
    full = np.concatenate([res.results[i]["out"] for i in range(N_CORES)], axis=0)
    return np.ascontiguousarray(full[:, :m_frames, :])


# revision 3
# speedup vs baseline: 2.6154x; 2.6154x over previous
"""Alignment generator (length regulator) on 8 TRN2 NeuronCores.

out[b, f, j] = 1.0  iff  starts[b,j] <= f < ends[b,j]  (ends = cumsum(dur))

Each output row out[b, f, :] is one-hot at token_id[b, f] =
searchsorted(ends[b], f, side='right') (or all-zero when no token covers
frame f). The host computes token_id from the tiny [32, 512] duration input;
each core then generates its 4-row slab of the ~256MB output with one DVE
tensor_scalar(is_equal) per [128, 512] tile and streams it out in big HWDGE
DMAs. The kernel is write-bandwidth bound (~358-390 GB/s per core to HBM).

Layout: partition p of row b covers a CONTIGUOUS frame span, so every
output DMA is linear in DRAM (20KB+ contiguous per partition-descriptor).

SDMA engine skew (trace-derived): the HW DGE splits one DMA's partition dim
EVENLY across engines -- engines_used = largest divisor of n_partitions
that is <= 16, engine k taking the k-th contiguous partition block. So a
128-partition DMA puts partitions 120-127 on engine 15, which on this part
is ~19% slower than the rest; with uniform spans it runs ~12us past the end
of the stream as a solo straggler. Fix: a 120-PARTITION main DMA (exactly
15 engines x 8 partitions, engine 15 idle; 15 engines x ~27 GB/s still
exceeds the per-core HBM cap) plus a tiny 8-partition DMA for partitions
120..127, which carry short L_e spans (8-partition DMAs land on engines
0-7). Partition counts must keep a large divisor <= 16: 92 partitions,
say, fan out to only 4 engines and run 3x slower end to end.

Raw Bass (no Tile): this walrus build only allows a single sync-wait per
compute/DMA instruction, so all synchronization is explicit standalone
wait_ge with a ring of NBUF buffers and one completion semaphore per buffer
slot; per-slot DMA-count bookkeeping makes "slot's previous DMAs fully
drained" provable from a 16*count threshold (every DMA increments its sem
by 16 regardless of partition count, cf. concourse/zero.py).

Sharding: pure data parallelism, batch dim 32 -> 4 rows per core; no
collectives.
"""

import math
from contextlib import ExitStack

import numpy as np

import concourse.bass as bass
import concourse.mybir as mybir
from concourse.bass_utils import run_bass_kernel_spmd

N_CORES = 8
B = 32          # batch
T = 512         # tokens
P = 128         # SBUF partitions
NMAIN = 120     # main partitions (engines 0-14); 120..127 are the light set
GROUP = 12      # max span steps per buffer slot / steady-state chunk
NBUF = 4        # output buffer ring slots

_nc_cache: dict[tuple[int, int], bass.Bass] = {}


def _geometry(m_frames: int):
    """Span lengths: partitions 0..119 get L_n frames, 120..127 get L_e.
    Minimize the per-engine load 8*L_n + L_e (engines 0-7 absorb the light
    spans), tie-break on padding."""
    best = None
    for L_n in range(max(1, math.ceil(m_frames / P)),
                     math.ceil(m_frames / NMAIN) + 2):
        L_e = max(0, math.ceil((m_frames - NMAIN * L_n) / 8))
        if L_e > L_n:
            continue
        m_pad = NMAIN * L_n + 8 * L_e
        cand = (8 * L_n + L_e, m_pad, L_n, L_e)
        if best is None or cand < best:
            best = cand
    _, m_pad, L_n, L_e = best
    assert m_pad >= m_frames
    return L_n, L_e, m_pad


def _chunks(L_n: int):
    """Split L_n span steps into near-equal chunks of at most GROUP."""
    n_ch = math.ceil(L_n / GROUP)
    base, rem = divmod(L_n, n_ch)
    return [base + 1] * rem + [base] * (n_ch - rem)


def _rounds(L_n: int, b_loc: int):
    """(row, first_span_step, n_span_steps). Ramp the first row's chunks
    (1,1,2,4,...) so the first output DMA is issued as soon as possible
    after the input lands -- the DMA stream is the bottleneck and every ns
    it starts earlier is a ns off the kernel."""
    sizes = _chunks(L_n)
    ramp, s = [], 0
    for x in [1, 1, 2, 4, 8, 16, 32, 64]:
        if s >= sizes[0]:
            break
        g = min(x, sizes[0] - s)
        ramp.append(g)
        s += g
    rounds = []
    for b in range(b_loc):
        g0 = 0
        for g in (ramp + sizes[1:]) if b == 0 else sizes:
            rounds.append((b, g0, g))
            g0 += g
    return rounds


def _build(m_frames: int, b_loc: int) -> bass.Bass:
    """Per-core Bass graph writing a [b_loc, m_pad, T] padded output slab."""
    L_n, L_e, m_pad = _geometry(m_frames)
    rounds = _rounds(L_n, b_loc)
    n_rounds = len(rounds)

    def light_g(g0, g):
        return max(0, min(g, L_e - g0))

    # DMA instructions per round: main always; light while the light span
    # (first L_e steps) overlaps this chunk.
    ndma = [1 + (1 if light_g(g0, g) > 0 else 0) for (_, g0, g) in rounds]

    nc = bass.Bass()
    # input column (b*L_n + k) on partition p = token id of frame
    # (span_start(p) + k) of row b; the iota row J (J[p,j] = j) is generated
    # on-device by GpSimd in parallel with this DMA
    tid = nc.declare_dram_parameter(
        "tid", [P, b_loc * L_n], mybir.dt.float32, isOutput=False
    )
    out = nc.declare_dram_parameter(
        "out", [b_loc, m_pad, T], mybir.dt.float32, isOutput=True
    )

    with ExitStack() as ctx:
        sb = ctx.enter_context(
            nc.sbuf_tensor("sb", [P, b_loc * L_n], mybir.dt.float32)
        )
        Jsb = ctx.enter_context(nc.sbuf_tensor("J", [P, T], mybir.dt.float32))
        bufs = [
            ctx.enter_context(
                nc.sbuf_tensor(f"buf{s}", [P, GROUP * T], mybir.dt.float32)
            )
            for s in range(NBUF)
        ]
        in_sem = ctx.enter_context(nc.semaphore("in_sem"))
        j_sem = ctx.enter_context(nc.semaphore("j_sem"))
        c_sem = ctx.enter_context(nc.semaphore("c_sem"))
        d_sems = [ctx.enter_context(nc.semaphore(f"d_sem{s}")) for s in range(NBUF)]
        block = ctx.enter_context(nc.Block())

        @block.gpsimd
        def _(gpsimd):
            # values 0..511 are exact in fp32
            gpsimd.iota(
                Jsb[:, :],
                pattern=[[1, T]],
                base=0,
                channel_multiplier=0,
                allow_small_or_imprecise_dtypes=True,
            ).then_inc(j_sem, 1)

        @block.sync
        def _(sync):
            sync.dma_start(out=sb[:, :], in_=tid[:, :]).then_inc(in_sem, 16)
            # main segment: partitions 0:120 -> engines 0-14, 8 each
            for r, (b, g0, g) in enumerate(rounds):
                s = r % NBUF
                sync.wait_ge(c_sem, r + 1)
                dview = out[b][: NMAIN * L_n].rearrange(
                    "(p i) t -> p (i t)", p=NMAIN
                )[:, g0 * T : (g0 + g) * T]
                sync.dma_start(
                    out=dview, in_=bufs[s][:NMAIN, : g * T]
                ).then_inc(d_sems[s], 16)
            # all output bytes landed before the NEFF may finish
            tot = [0] * NBUF
            for r in range(n_rounds):
                tot[r % NBUF] += ndma[r]
            for s in range(NBUF):
                if tot[s]:
                    sync.wait_ge(d_sems[s], 16 * tot[s])

        @block.scalar
        def _(scalar):
            # light segment: partitions 120:128 -> engines 0-7, 1 each
            for r, (b, g0, g) in enumerate(rounds):
                s = r % NBUF
                gl = light_g(g0, g)
                if gl <= 0:
                    continue
                scalar.wait_ge(c_sem, r + 1)
                dview = out[b][NMAIN * L_n : NMAIN * L_n + 8 * L_e].rearrange(
                    "(p i) t -> p (i t)", p=8
                )[:, g0 * T : (g0 + gl) * T]
                scalar.dma_start(
                    out=dview, in_=bufs[s][NMAIN:, : gl * T]
                ).then_inc(d_sems[s], 16)

        @block.vector
        def _(vector):
            vector.wait_ge(j_sem, 1)
            vector.wait_ge(in_sem, 16)
            cum = [0] * NBUF  # DMAs issued into slot s before this round
            for r, (b, g0, g) in enumerate(rounds):
                s = r % NBUF
                if r >= NBUF:
                    # slot's previous DMAs (round r-NBUF) fully drained
                    vector.wait_ge(d_sems[s], 16 * cum[s])
                last = None
                for k in range(g):
                    col = b * L_n + g0 + k
                    last = nc.vector.tensor_scalar(
                        out=bufs[s][:, k * T : (k + 1) * T],
                        in0=Jsb[:, :],
                        scalar1=sb[:, col : col + 1],
                        scalar2=None,
                        op0=mybir.AluOpType.is_equal,
                    )
                last.then_inc(c_sem, 1)
                cum[s] += ndma[r]

    return nc


def _token_ids(dur: np.ndarray, m_pad: int) -> np.ndarray:
    """tid[b, f] = index of the token whose frame interval contains f,
    or T (out of range -> all-zero output row) when no token covers f."""
    ends = np.cumsum(dur.astype(np.int64), axis=1)
    frames = np.arange(m_pad, dtype=np.int64)
    tid = np.empty((dur.shape[0], m_pad), dtype=np.float32)
    for b in range(dur.shape[0]):
        tid[b] = np.searchsorted(ends[b], frames, side="right")
    return tid


def _prepare(duration_predictor_output: np.ndarray, max_frames):
    """Host-side prep: token ids, per-core input maps, cached Bass graph."""
    dur = np.asarray(duration_predictor_output)
    m_frames = int(max_frames)
    b_loc = B // N_CORES
    L_n, L_e, m_pad = _geometry(m_frames)

    tid = _token_ids(dur, m_pad)  # [B, m_pad] float32

    key = (m_frames, b_loc)
    nc = _nc_cache.get(key)
    if nc is None:
        nc = _build(m_frames, b_loc)
        _nc_cache[key] = nc

    # partition p's span start / length in the padded frame space
    ps = np.arange(P)
    starts = np.where(ps < NMAIN, ps * L_n, NMAIN * L_n + (ps - NMAIN) * L_e)
    span_len = np.where(ps < NMAIN, L_n, L_e)
    # gather index [P, L_n]: frame for (partition, span step), clamped;
    # steps beyond a light partition's span are masked to token id T
    k = np.arange(L_n)[None, :]
    idx = np.minimum(starts[:, None] + k, m_pad - 1)
    mask = k >= span_len[:, None]

    in_maps = []
    for i in range(N_CORES):
        cols = []
        for b in range(b_loc):
            tb = tid[i * b_loc + b][idx]          # [P, L_n]
            tb[mask] = float(T)
            cols.append(tb)
        in_maps.append({"tid": np.ascontiguousarray(np.concatenate(cols, axis=1))})
    return nc, in_maps


def kernel(duration_predictor_output: np.ndarray, max_frames) -> np.ndarray:
    dur = np.asarray(duration_predictor_output)
    m_frames = int(max_frames)
    if m_frames <= 0:
        return np.zeros((dur.shape[0], 0, dur.shape[1]), dtype=np.float32)

    nc, in_maps = _prepare(dur, m_frames)
    res = run_bass_kernel_spmd(nc, in_maps, core_ids=list(range(N_CORES)))
    full = np.concatenate([res.results[i]["out"] for i in range(N_CORES)], axis=0)
    return np.ascontiguousarray(full[:, :m_frames, :])


# revision 5
# speedup vs baseline: 2.7386x; 1.0471x over previous
"""Alignment generator (length regulator) on 8 TRN2 NeuronCores.

out[b, f, j] = 1.0  iff  starts[b,j] <= f < ends[b,j]  (ends = cumsum(dur))

Each output row out[b, f, :] is one-hot at token_id[b, f] =
searchsorted(ends[b], f, side='right') (or all-zero when no token covers
frame f). The host computes token_id from the tiny [32, 512] duration input;
each core then generates its 4-row slab of the ~256MB output with one DVE
tensor_scalar(is_equal) per [128, 512] tile and streams it out in big HWDGE
DMAs. The kernel is write-bandwidth bound (~358-390 GB/s per core to HBM).

Layout: partition p of row b covers a CONTIGUOUS frame span, so every
output DMA is linear in DRAM (20KB+ contiguous per partition-descriptor).

SDMA engine skew (trace-derived): the HW DGE splits one DMA's partition dim
EVENLY across engines -- engines_used = largest divisor of n_partitions
that is <= 16, engine k taking the k-th contiguous partition block. So a
128-partition DMA puts partitions 120-127 on engine 15, which on this part
is ~19% slower than the rest; with uniform spans it runs ~12us past the end
of the stream as a solo straggler. Fix: a 120-PARTITION main DMA (exactly
15 engines x 8 partitions, engine 15 idle; 15 engines x ~27 GB/s still
exceeds the per-core HBM cap) plus a tiny 8-partition DMA for partitions
120..127, which carry short L_e spans (8-partition DMAs land on engines
0-7). Partition counts must keep a large divisor <= 16: 92 partitions,
say, fan out to only 4 engines and run 3x slower end to end.

Raw Bass (no Tile): this walrus build only allows a single sync-wait per
compute/DMA instruction, so all synchronization is explicit standalone
wait_ge with a ring of NBUF buffers and one completion semaphore per buffer
slot; per-slot DMA-count bookkeeping makes "slot's previous DMAs fully
drained" provable from a 16*count threshold (every DMA increments its sem
by 16 regardless of partition count, cf. concourse/zero.py).

Sharding: pure data parallelism, batch dim 32 -> 4 rows per core; no
collectives.
"""

import math
from contextlib import ExitStack

import numpy as np

import concourse.bass as bass
import concourse.mybir as mybir
from concourse.bass_utils import run_bass_kernel_spmd

N_CORES = 8
B = 32          # batch
T = 512         # tokens
P = 128         # SBUF partitions
NMAIN = 120     # main partitions (engines 0-14); 120..127 are the light set
GROUP = 8       # span steps per chunk: 8*T*4B = 16KB per partition, the
                # largest single-descriptor size (bigger chunks get split
                # into 8KB descriptors, dropping engines to ~22.5 GB/s)
NBUF = 4        # output buffer ring slots

_nc_cache: dict[tuple[int, int], bass.Bass] = {}


def _geometry(m_frames: int):
    """Span lengths: partitions 0..119 get L_n frames, 120..127 get L_e.
    Minimize the per-engine load 8*L_n + L_e (engines 0-7 absorb the light
    spans), tie-break on padding."""
    best = None
    for L_n in range(max(1, math.ceil(m_frames / P)),
                     math.ceil(m_frames / NMAIN) + 2):
        L_e = max(0, math.ceil((m_frames - NMAIN * L_n) / 8))
        if L_e > L_n:
            continue
        m_pad = NMAIN * L_n + 8 * L_e
        cand = (8 * L_n + L_e, m_pad, L_n, L_e)
        if best is None or cand < best:
            best = cand
    _, m_pad, L_n, L_e = best
    assert m_pad >= m_frames
    return L_n, L_e, m_pad


def _chunks(L_n: int):
    """Split L_n span steps into chunks of exactly GROUP (full 16KB
    descriptors) plus one remainder chunk."""
    sizes = [GROUP] * (L_n // GROUP)
    if L_n % GROUP:
        sizes.append(L_n % GROUP)
    return sizes


def _rounds(L_n: int, b_loc: int):
    """(row, first_span_step, n_span_steps). Ramp the first row's chunks
    (1,1,2,4,...) so the first output DMA is issued as soon as possible
    after the input lands -- the DMA stream is the bottleneck and every ns
    it starts earlier is a ns off the kernel."""
    sizes = _chunks(L_n)
    ramp, s = [], 0
    for x in [1, 1, 2, 4, 8, 16, 32, 64]:
        if s >= sizes[0]:
            break
        g = min(x, sizes[0] - s)
        ramp.append(g)
        s += g
    rounds = []
    for b in range(b_loc):
        g0 = 0
        for g in (ramp + sizes[1:]) if b == 0 else sizes:
            rounds.append((b, g0, g))
            g0 += g
    return rounds


def _build(m_frames: int, b_loc: int) -> bass.Bass:
    """Per-core Bass graph writing a [b_loc, m_pad, T] padded output slab."""
    L_n, L_e, m_pad = _geometry(m_frames)
    rounds = _rounds(L_n, b_loc)
    n_rounds = len(rounds)

    def light_g(g0, g):
        return max(0, min(g, L_e - g0))

    # DMA instructions per round: main always; light while the light span
    # (first L_e steps) overlaps this chunk.
    ndma = [1 + (1 if light_g(g0, g) > 0 else 0) for (_, g0, g) in rounds]

    nc = bass.Bass()
    # input column (b*L_n + k) on partition p = token id of frame
    # (span_start(p) + k) of row b; the iota row J (J[p,j] = j) is generated
    # on-device by GpSimd in parallel with this DMA
    tid = nc.declare_dram_parameter(
        "tid", [P, b_loc * L_n], mybir.dt.float32, isOutput=False
    )
    out = nc.declare_dram_parameter(
        "out", [b_loc, m_pad, T], mybir.dt.float32, isOutput=True
    )

    with ExitStack() as ctx:
        sb = ctx.enter_context(
            nc.sbuf_tensor("sb", [P, b_loc * L_n], mybir.dt.float32)
        )
        Jsb = ctx.enter_context(nc.sbuf_tensor("J", [P, T], mybir.dt.float32))
        bufs = [
            ctx.enter_context(
                nc.sbuf_tensor(f"buf{s}", [P, GROUP * T], mybir.dt.float32)
            )
            for s in range(NBUF)
        ]
        in_sem = ctx.enter_context(nc.semaphore("in_sem"))
        j_sem = ctx.enter_context(nc.semaphore("j_sem"))
        c_sem = ctx.enter_context(nc.semaphore("c_sem"))
        d_sems = [ctx.enter_context(nc.semaphore(f"d_sem{s}")) for s in range(NBUF)]
        block = ctx.enter_context(nc.Block())

        @block.gpsimd
        def _(gpsimd):
            # values 0..511 are exact in fp32
            gpsimd.iota(
                Jsb[:, :],
                pattern=[[1, T]],
                base=0,
                channel_multiplier=0,
                allow_small_or_imprecise_dtypes=True,
            ).then_inc(j_sem, 1)

        @block.sync
        def _(sync):
            sync.dma_start(out=sb[:, :], in_=tid[:, :]).then_inc(in_sem, 16)
            # main segment: partitions 0:120 -> engines 0-14, 8 each
            for r, (b, g0, g) in enumerate(rounds):
                s = r % NBUF
                sync.wait_ge(c_sem, r + 1)
                dview = out[b][: NMAIN * L_n].rearrange(
                    "(p i) t -> p (i t)", p=NMAIN
                )[:, g0 * T : (g0 + g) * T]
                sync.dma_start(
                    out=dview, in_=bufs[s][:NMAIN, : g * T]
                ).then_inc(d_sems[s], 16)
            # all output bytes landed before the NEFF may finish
            tot = [0] * NBUF
            for r in range(n_rounds):
                tot[r % NBUF] += ndma[r]
            for s in range(NBUF):
                if tot[s]:
                    sync.wait_ge(d_sems[s], 16 * tot[s])

        @block.scalar
        def _(scalar):
            # light segment: partitions 120:128 -> engines 0-7, 1 each
            for r, (b, g0, g) in enumerate(rounds):
                s = r % NBUF
                gl = light_g(g0, g)
                if gl <= 0:
                    continue
                scalar.wait_ge(c_sem, r + 1)
                dview = out[b][NMAIN * L_n : NMAIN * L_n + 8 * L_e].rearrange(
                    "(p i) t -> p (i t)", p=8
                )[:, g0 * T : (g0 + gl) * T]
                scalar.dma_start(
                    out=dview, in_=bufs[s][NMAIN:, : gl * T]
                ).then_inc(d_sems[s], 16)

        @block.vector
        def _(vector):
            vector.wait_ge(j_sem, 1)
            vector.wait_ge(in_sem, 16)
            cum = [0] * NBUF  # DMAs issued into slot s before this round
            for r, (b, g0, g) in enumerate(rounds):
                s = r % NBUF
                if r >= NBUF:
                    # slot's previous DMAs (round r-NBUF) fully drained
                    vector.wait_ge(d_sems[s], 16 * cum[s])
                last = None
                for k in range(g):
                    col = b * L_n + g0 + k
                    last = nc.vector.tensor_scalar(
                        out=bufs[s][:, k * T : (k + 1) * T],
                        in0=Jsb[:, :],
                        scalar1=sb[:, col : col + 1],
                        scalar2=None,
                        op0=mybir.AluOpType.is_equal,
                    )
                last.then_inc(c_sem, 1)
                cum[s] += ndma[r]

    return nc


def _token_ids(dur: np.ndarray, m_pad: int) -> np.ndarray:
    """tid[b, f] = index of the token whose frame interval contains f,
    or T (out of range -> all-zero output row) when no token covers f."""
    ends = np.cumsum(dur.astype(np.int64), axis=1)
    frames = np.arange(m_pad, dtype=np.int64)
    tid = np.empty((dur.shape[0], m_pad), dtype=np.float32)
    for b in range(dur.shape[0]):
        tid[b] = np.searchsorted(ends[b], frames, side="right")
    return tid


def _prepare(duration_predictor_output: np.ndarray, max_frames):
    """Host-side prep: token ids, per-core input maps, cached Bass graph."""
    dur = np.asarray(duration_predictor_output)
    m_frames = int(max_frames)
    b_loc = B // N_CORES
    L_n, L_e, m_pad = _geometry(m_frames)

    tid = _token_ids(dur, m_pad)  # [B, m_pad] float32

    key = (m_frames, b_loc)
    nc = _nc_cache.get(key)
    if nc is None:
        nc = _build(m_frames, b_loc)
        _nc_cache[key] = nc

    # partition p's span start / length in the padded frame space
    ps = np.arange(P)
    starts = np.where(ps < NMAIN, ps * L_n, NMAIN * L_n + (ps - NMAIN) * L_e)
    span_len = np.where(ps < NMAIN, L_n, L_e)
    # gather index [P, L_n]: frame for (partition, span step), clamped;
    # steps beyond a light partition's span are masked to token id T
    k = np.arange(L_n)[None, :]
    idx = np.minimum(starts[:, None] + k, m_pad - 1)
    mask = k >= span_len[:, None]

    in_maps = []
    for i in range(N_CORES):
        cols = []
        for b in range(b_loc):
            tb = tid[i * b_loc + b][idx]          # [P, L_n]
            tb[mask] = float(T)
            cols.append(tb)
        in_maps.append({"tid": np.ascontiguousarray(np.concatenate(cols, axis=1))})
    return nc, in_maps


def kernel(duration_predictor_output: np.ndarray, max_frames) -> np.ndarray:
    dur = np.asarray(duration_predictor_output)
    m_frames = int(max_frames)
    if m_frames <= 0:
        return np.zeros((dur.shape[0], 0, dur.shape[1]), dtype=np.float32)

    nc, in_maps = _prepare(dur, m_frames)
    res = run_bass_kernel_spmd(nc, in_maps, core_ids=list(range(N_CORES)))
    full = np.concatenate([res.results[i]["out"] for i in range(N_CORES)], axis=0)
    return np.ascontiguousarray(full[:, :m_frames, :])
